# revision 1
# baseline (speedup 1.0000x reference)
"""CrossAttention Trainium2 kernel.

Problem: nn_CrossAttention (B=4, N=M=1024, DIM=CTX_DIM=1024, H=16, DH=64).

Sharding: 8 cores = batch (4) x head-group (2 groups of 8 heads).
Each core computes, for its (b, g):
    q = rope(x[b] @ Wq[:, g])
    k = rope(context[b] @ Wk[:, g]);  v = context[b] @ Wv[:, g]
    attn = softmax(q k^T / sqrt(dh))     (mask is all-ones by construction)
    partial_out[b,g] = (attn @ v) @ Wout[g, :]
Host transposes x/context per batch (input marshalling), sums the two
head-group partials per batch, and adds bout.

Device layouts (contraction dims on SBUF partitions):
    xT/ctxT  [128, 8, 1024]  (dim-chunk on partitions)  DMA'd from host-side T
    qT/kT    [128, 4, 1024]  (inner col on partitions; head h -> rows (h%2)*64,
                              tile index h//2)
    v        [128, 8, 65]    per m-chunk; col 64 = 1.0 (softmax-denominator trick)
    expT     [128, 1024]     per (head, m-chunk): exp(scale * k q^T), m on partitions
    attn@V   psum [65, n]    row 64 accumulates the softmax denominator
All matmul operands are float32r-typed (TF32-like, 1 cycle/row at N=512) with
fp32 PSUM accumulation; walrus requires producers to declare f32r outputs.

Softmax denominators: ones-column of v gives sums in psum row 64; the row is
reshaped to [8, 128] by DMA so one cheap lane-parallel DVE reciprocal covers a
whole head, then bounced through DRAM to broadcast across the head's 64
partitions (SBUF partition-step-0 reads are illegal). The normalize multiply
is deferred one head to keep the DVE queue from stalling on the broadcast.

SBUF pool lifetimes are stacked: xT/ctxT (64KB/partition) are freed after the
projections, making room for a 16-deep f32r exp-tile pool in the attention
phase.
"""

import os
import numpy as np

B, N, M = 4, 1024, 1024
DIM = 1024
H, DH = 16, 64
ISH = 512  # inner shard per core (8 heads * 64)
SCALE = DH ** -0.5
P = 128

_CACHE = {}
_LAST_EXEC_NS = None


def _build_program():
    from contextlib import ExitStack

    import concourse.tile as tile
    from concourse import bacc, mybir

    f32 = mybir.dt.float32
    f32r = mybir.dt.float32r
    Exp = mybir.ActivationFunctionType.Exp

    nc = bacc.Bacc("TRN2", target_bir_lowering=False, debug=False, num_devices=8)

    xbT = nc.dram_tensor("xbT", [DIM, N], f32r, kind="ExternalInput").ap()
    cxT = nc.dram_tensor("cxT", [DIM, M], f32r, kind="ExternalInput").ap()
    wq = nc.dram_tensor("wq", [DIM, ISH], f32r, kind="ExternalInput").ap()
    wk = nc.dram_tensor("wk", [DIM, ISH], f32r, kind="ExternalInput").ap()
    wv = nc.dram_tensor("wv", [DIM, ISH], f32r, kind="ExternalInput").ap()
    wo = nc.dram_tensor("wo", [ISH, DIM], f32r, kind="ExternalInput").ap()
    cos2 = nc.dram_tensor("cos2", [P, N], f32, kind="ExternalInput").ap()
    sin2 = nc.dram_tensor("sin2", [P, N], f32, kind="ExternalInput").ap()
    out = nc.dram_tensor("out", [N, DIM], f32, kind="ExternalOutput").ap()

    with tile.TileContext(nc) as tc, ExitStack() as ctx:
        const = ctx.enter_context(tc.tile_pool(name="const", bufs=1))
        wpool = ctx.enter_context(tc.tile_pool(name="wpool", bufs=2))
        qk = ctx.enter_context(tc.tile_pool(name="qk", bufs=1))
        vpool = ctx.enter_context(tc.tile_pool(name="vpool", bufs=8))
        drp = ctx.enter_context(tc.tile_pool(name="drp", bufs=4, space="DRAM"))
        psmm = ctx.enter_context(tc.tile_pool(name="psmm", bufs=6, space="PSUM"))
        psav = ctx.enter_context(tc.tile_pool(name="psav", bufs=2, space="PSUM"))

        ones_sb = const.tile([P, 8], f32, tag="ones")
        nc.vector.memset(ones_sb[:], 1.0)
        cos_sb = const.tile([P, N], f32, tag="cos")
        nc.gpsimd.dma_start(cos_sb[:], cos2)
        sin_sb = const.tile([P, N], f32, tag="sin")
        nc.gpsimd.dma_start(sin_sb[:], sin2)

        # ---- phase A: projections (xT/ctxT big tiles live only here)
        with tc.tile_pool(name="bigT", bufs=2) as bigT, \
                tc.tile_pool(name="tmpp", bufs=2) as tmpp:

            def load_T(srcT):
                t = bigT.tile([P, 8, N], f32r, tag="bigT")
                for k in range(8):
                    nc.sync.dma_start(t[:, k, :], srcT[k * P:(k + 1) * P, :])
                return t

            def rope_copyback(ps, dst, nsl):
                """dst = ps * cos + rotate_half(ps) * sin_signed (ps in PSUM)."""
                tmp = tmpp.tile([P, 512], f32, tag="tmp")
                for blk in range(4):
                    d0 = blk * 32
                    s0 = (blk ^ 1) * 32
                    nc.vector.tensor_mul(
                        out=tmp[d0:d0 + 32, :],
                        in0=ps[s0:s0 + 32, :],
                        in1=sin_sb[d0:d0 + 32, nsl],
                    )
                nc.vector.tensor_mul(out=dst, in0=ps[:], in1=cos_sb[:, nsl])
                nc.vector.tensor_add(out=dst, in0=dst, in1=tmp[:])

            def project_rope(xT, w_dram, tag):
                w_sb = wpool.tile([P, 8, ISH], f32r, tag="w")
                for k in range(8):
                    nc.scalar.dma_start(w_sb[:, k, :], w_dram[k * P:(k + 1) * P, :])
                dst = qk.tile([P, 4, N], f32r, tag=tag)
                for ic in range(4):
                    pss = [psmm.tile([P, 512], f32, tag="mm", name=f"ps{_i}")
                           for _i in range(2)]
                    for k in range(8):
                        for ns in range(2):
                            nc.tensor.matmul(
                                pss[ns][:],
                                lhsT=w_sb[:, k, ic * P:(ic + 1) * P],
                                rhs=xT[:, k, ns * 512:(ns + 1) * 512],
                                start=(k == 0),
                                stop=(k == 7),
                            )
                    for ns in range(2):
                        nsl = slice(ns * 512, (ns + 1) * 512)
                        rope_copyback(pss[ns], dst[:, ic, nsl], nsl)
                return dst

            xT = load_T(xbT)
            qT = project_rope(xT, wq, "qT")
            cT = load_T(cxT)
            kT = project_rope(cT, wk, "kT")

            wv_sb = wpool.tile([P, 8, ISH], f32r, tag="w")
            for k in range(8):
                nc.gpsimd.dma_start(wv_sb[:, k, :], wv[k * P:(k + 1) * P, :])
            vsb = []
            for mch in range(8):
                ps = psmm.tile([P, 512], f32, tag="mm")
                for k in range(8):
                    nc.tensor.matmul(
                        ps[:],
                        lhsT=cT[:, k, mch * P:(mch + 1) * P],
                        rhs=wv_sb[:, k, :],
                        start=(k == 0),
                        stop=(k == 7),
                    )
                vt = vpool.tile([P, 8, DH + 1], f32r, tag="v")
                nc.any.tensor_copy(
                    out=vt[:, :, 0:DH], in_=ps.rearrange("p (h d) -> p h d", d=DH)
                )
                nc.any.tensor_copy(out=vt[:, :, DH], in_=ones_sb[:])
                vsb.append(vt)

        # ---- phase B: attention + final projection (bigT space now free)
        epool = ctx.enter_context(tc.tile_pool(name="epool", bufs=16))
        recp = ctx.enter_context(tc.tile_pool(name="recp", bufs=2))
        sumsp = ctx.enter_context(tc.tile_pool(name="sumsp", bufs=2))
        rbcp = ctx.enter_context(tc.tile_pool(name="rbcp", bufs=2))
        opool = ctx.enter_context(tc.tile_pool(name="opool", bufs=4))

        def dots_exp(h):
            t2, r0 = h // 2, (h % 2) * 64
            qh = qT[r0:r0 + 64, t2, :]
            kh = kT[r0:r0 + 64, t2, :]
            es = []
            for mch in range(8):
                e = epool.tile([P, N], f32r, tag="e")
                for ns in range(2):
                    psd = psmm.tile([P, 512], f32, tag="mm")
                    nc.tensor.matmul(
                        psd[:],
                        lhsT=kh[:, mch * P:(mch + 1) * P],
                        rhs=qh[:, ns * 512:(ns + 1) * 512],
                        start=True,
                        stop=True,
                    )
                    nc.scalar.activation(
                        e[:, ns * 512:(ns + 1) * 512], psd[:], Exp, scale=SCALE
                    )
                es.append(e)
            return es

        aoT = qk.tile([P, 4, N], f32r, tag="aoT")

        def attn_v(h, es):
            t2, r0 = h // 2, (h % 2) * 64
            pos = [psav.tile([DH + 1, 512], f32, tag="av", name=f"po{_i}")
                   for _i in range(2)]
            for mch in range(8):
                for ns in range(2):
                    nc.tensor.matmul(
                        pos[ns][:],
                        lhsT=vsb[mch][:, h, :],
                        rhs=es[mch][:, ns * 512:(ns + 1) * 512],
                        start=(mch == 0),
                        stop=(mch == 7),
                    )
            srow = recp.tile([DH + 1, N], f32, tag="srow")
            for ns in range(2):
                nsl = slice(ns * 512, (ns + 1) * 512)
                po = pos[ns]
                nc.vector.tensor_copy(out=aoT[r0:r0 + 64, t2, nsl], in_=po[0:64, :])
                nc.vector.tensor_copy(out=srow[DH:DH + 1, nsl], in_=po[DH:DH + 1, :])
            # reciprocal of the denominators, lane-parallel via DMA reshape,
            # broadcast across the head's 64 partitions via a DRAM bounce
            st = sumsp.tile([8, P], f32, tag="st")
            nc.sync.dma_start(st[:], srow[DH:DH + 1, :])
            rt = sumsp.tile([8, P], f32, tag="rt")
            nc.vector.reciprocal(out=rt[:], in_=st[:])
            rd = drp.tile([N], f32, tag="rd")
            nc.sync.dma_start(rd[:], rt[:])
            rb = rbcp.tile([P, N], f32, tag="rb")
            nc.sync.dma_start(rb[r0:r0 + 64, :], rd[None, :].to_broadcast((64, N)))
            ao = aoT[r0:r0 + 64, t2, :]

            def _mult(ao=ao, rb=rb, r0=r0):
                nc.vector.tensor_mul(out=ao, in0=ao, in1=rb[r0:r0 + 64, :])
            return _mult

        es_cur = dots_exp(0)
        pending_mult = None
        for h in range(8):
            es_next = dots_exp(h + 1) if h < 7 else None
            m = attn_v(h, es_cur)
            if pending_mult is not None:
                pending_mult()
            pending_mult = m
            es_cur = es_next
        pending_mult()

        # ---- final projection
        wo_sb = wpool.tile([P, 4, DIM], f32r, tag="w")
        for k in range(4):
            nc.scalar.dma_start(wo_sb[:, k, :], wo[k * P:(k + 1) * P, :])
        for nch in range(8):
            pfs = [psmm.tile([P, 512], f32, tag="mm", name=f"pf{_i}")
                   for _i in range(2)]
            for kc in range(4):
                for cc in range(2):
                    nc.tensor.matmul(
                        pfs[cc][:],
                        lhsT=aoT[:, kc, nch * P:(nch + 1) * P],
                        rhs=wo_sb[:, kc, cc * 512:(cc + 1) * 512],
                        start=(kc == 0),
                        stop=(kc == 3),
                    )
            for cc in range(2):
                ot = opool.tile([P, 512], f32, tag="o")
                nc.any.tensor_copy(out=ot[:], in_=pfs[cc][:])
                eng = nc.scalar if cc else nc.sync
                eng.dma_start(
                    out[nch * P:(nch + 1) * P, cc * 512:(cc + 1) * 512], ot[:]
                )

    nc.compile()
    return nc


def _get_program():
    if "nc" not in _CACHE:
        _CACHE["nc"] = _build_program()
    return _CACHE["nc"]


def make_in_maps(x, context, rotary_pos, Wq, Wkv, Wout):
    x = np.asarray(x, dtype=np.float32)
    context = np.asarray(context, dtype=np.float32)
    rotary_pos = np.asarray(rotary_pos, dtype=np.float32)
    Wq = np.asarray(Wq, dtype=np.float32)
    Wkv = np.asarray(Wkv, dtype=np.float32)
    Wout = np.asarray(Wout, dtype=np.float32)

    cosT = np.ascontiguousarray(np.cos(rotary_pos).T)  # [64, 1024]
    sinT = np.sin(rotary_pos).T
    sin_signed = np.concatenate([-sinT[:32], sinT[32:]], axis=0)
    cos2 = np.ascontiguousarray(np.vstack([cosT, cosT]))
    sin2 = np.ascontiguousarray(np.vstack([sin_signed, sin_signed]))

    in_maps = []
    for core in range(8):
        b, g = core // 2, core % 2
        cs = slice(g * ISH, (g + 1) * ISH)
        in_maps.append({
            "xbT": np.ascontiguousarray(x[b].T),
            "cxT": np.ascontiguousarray(context[b].T),
            "wq": np.ascontiguousarray(Wq[:, cs]),
            "wk": np.ascontiguousarray(Wkv[:, g * ISH:(g + 1) * ISH]),
            "wv": np.ascontiguousarray(Wkv[:, H * DH + g * ISH:H * DH + (g + 1) * ISH]),
            "wo": np.ascontiguousarray(Wout[cs, :]),
            "cos2": cos2,
            "sin2": sin2,
        })
    return in_maps


def kernel(x, context, mask, context_mask, rotary_pos, Wq, Wkv, Wout, bout):
    global _LAST_EXEC_NS
    from concourse.bass_utils import run_bass_kernel_spmd

    nc = _get_program()
    in_maps = make_in_maps(x, context, rotary_pos, Wq, Wkv, Wout)

    trace = bool(os.environ.get("BASS_KERNEL_TRACE"))
    res = run_bass_kernel_spmd(nc, in_maps, core_ids=list(range(8)), trace=trace)
    _LAST_EXEC_NS = res.exec_time_ns
    _CACHE["last_results"] = res

    bout = np.asarray(bout, dtype=np.float32)
    full = np.empty((B, N, DIM), dtype=np.float32)
    for b in range(B):
        full[b] = res.results[2 * b]["out"] + res.results[2 * b + 1]["out"] + bout
    return full



# revision 34
# speedup vs baseline: 1.2033x; 1.2033x over previous
"""CrossAttention Trainium2 kernel (v2, all-bf16 single-pass pipeline).

Problem: nn_CrossAttention (B=4, N=M=1024, DIM=CTX_DIM=1024, H=16, DH=64).

Sharding: 8 cores = batch (4) x head-group (2 groups of 8 heads).
Each core computes, for its (b, g):
    q = rope(x[b] @ Wq[:, g])
    k = rope(context[b] @ Wk[:, g]);  v = context[b] @ Wv[:, g]
    attn = softmax(q k^T / sqrt(dh))     (mask is all-ones by construction)
    partial_out[b,g] = (attn @ v) @ Wout[g, :]
Host transposes x/context per batch, casts everything to bf16, sums the two
head-group partials per batch in fp32, and adds bout.

All tensors bf16 on SBUF (fp32 PSUM accumulation). Cost-model-driven layout:
engine time is free-dim-size only, DVE gets 2x for all-SBUF bf16 ops, PSUM
reads run 1x, activation exp is dtype-agnostic. Therefore:
  - dots psum tiles are 2-bank [128,1024] wide so each exp covers 1024 cols
  - rope copies PSUM->SBUF bf16 once (1x), then runs the 4 rotate-strip muls
    and cos-mul in bf16 SBUF (2x); the final add runs on the idle GPSIMD pool
  - softmax denominators: ones-column in V accumulates sum(exp) in psum row
    64; reciprocal runs directly on that psum row, then a DRAM bounce
    broadcasts it across the head's 64 partitions
  - out-projection is split: head-pairs 0-2 are projected mid-attention into
    bf16 SBUF partials, the tail only runs head-pair 3 + one add per n-chunk
"""

import os
import numpy as np

B, N, M = 4, 1024, 1024
DIM = 1024
H, DH = 16, 64
ISH = 512  # inner shard per core (8 heads * 64)
SCALE = DH ** -0.5
P = 128

_CACHE = {}
_LAST_EXEC_NS = None


def _build_program():
    from contextlib import ExitStack

    import concourse.tile as tile
    from concourse import bacc, mybir

    f32 = mybir.dt.float32
    bf16 = mybir.dt.bfloat16
    Exp = mybir.ActivationFunctionType.Exp

    nc = bacc.Bacc("TRN2", target_bir_lowering=False, debug=False, num_devices=8)

    xbT = nc.dram_tensor("xbT", [DIM, N], bf16, kind="ExternalInput").ap()
    cxT = nc.dram_tensor("cxT", [DIM, M], bf16, kind="ExternalInput").ap()
    wq = nc.dram_tensor("wq", [DIM, ISH], bf16, kind="ExternalInput").ap()
    wk = nc.dram_tensor("wk", [DIM, ISH], bf16, kind="ExternalInput").ap()
    wv = nc.dram_tensor("wv", [DIM, ISH], bf16, kind="ExternalInput").ap()
    wo = nc.dram_tensor("wo", [ISH, DIM], bf16, kind="ExternalInput").ap()
    cos2 = nc.dram_tensor("cos2", [P, N], bf16, kind="ExternalInput").ap()
    sin2 = nc.dram_tensor("sin2", [P, N], bf16, kind="ExternalInput").ap()
    out = nc.dram_tensor("out", [N, DIM], bf16, kind="ExternalOutput").ap()

    with tile.TileContext(nc) as tc, ExitStack() as ctx:
        const = ctx.enter_context(tc.tile_pool(name="const", bufs=1))
        wpool = ctx.enter_context(tc.tile_pool(name="wpool", bufs=1))
        xpool = ctx.enter_context(tc.tile_pool(name="xpool", bufs=1))
        qk = ctx.enter_context(tc.tile_pool(name="qk", bufs=1))
        qbp = ctx.enter_context(tc.tile_pool(name="qbp", bufs=3))
        tmpp = ctx.enter_context(tc.tile_pool(name="tmpp", bufs=3))
        vpool = ctx.enter_context(tc.tile_pool(name="vpool", bufs=8))
        epool = ctx.enter_context(tc.tile_pool(name="epool", bufs=24))
        rrp = ctx.enter_context(tc.tile_pool(name="rrp", bufs=2))
        rbcp = ctx.enter_context(tc.tile_pool(name="rbcp", bufs=2))
        osbp = ctx.enter_context(tc.tile_pool(name="osbp", bufs=8))
        ofp = ctx.enter_context(tc.tile_pool(name="ofp", bufs=4))
        psP = ctx.enter_context(tc.tile_pool(name="psP", bufs=2, space="PSUM"))
        psD = ctx.enter_context(tc.tile_pool(name="psD", bufs=2, space="PSUM"))
        psB = ctx.enter_context(tc.tile_pool(name="psB", bufs=2, space="PSUM"))

        cos_sb = const.tile([P, N], bf16, tag="cos")
        sin_sb = const.tile([P, N], bf16, tag="sin")

        # ---- input loads: one big rearranged DMA per tensor (per-DMA fixed
        # overheads dominate chunked loads; every projection contracts over
        # all 8 chunks anyway so chunk-granular gating buys nothing).
        # preload the Exp activation table while DMAs run (takes 1.3us; off
        # the first-exp critical path this way)
        dummy = const.tile([1, 8], f32, tag="dummy")
        nc.vector.memset(dummy[:], 0.0)
        nc.scalar.activation(dummy[:], dummy[:], Exp, scale=1.0)

        wq_sb = wpool.tile([P, 8, ISH], bf16, tag="wq")
        nc.sync.dma_start(wq_sb[:], wq.rearrange("(k p) m -> p k m", p=P))
        xT = xpool.tile([P, 8, N], bf16, tag="xT")
        nc.sync.dma_start(xT[:], xbT.rearrange("(k p) n -> p k n", p=P))
        wk_sb = wpool.tile([P, 8, ISH], bf16, tag="wk")
        nc.sync.dma_start(wk_sb[:], wk.rearrange("(k p) m -> p k m", p=P))
        cT = xpool.tile([P, 8, N], bf16, tag="cT")
        nc.sync.dma_start(cT[:], cxT.rearrange("(k p) n -> p k n", p=P))
        # smaller late-needed loads issued after the critical four so their
        # transfers don't delay x/ctx on the (serialized) DMA engines
        nc.gpsimd.dma_start(cos_sb[:], cos2)
        nc.gpsimd.dma_start(sin_sb[:], sin2)
        wv_sb = wpool.tile([P, 8, ISH], bf16, tag="wv")
        nc.gpsimd.dma_start(wv_sb[:], wv.rearrange("(k p) m -> p k m", p=P))
        wo_sb = wpool.tile([P, 4, DIM], bf16, tag="wo")
        nc.gpsimd.dma_start(wo_sb[:], wo.rearrange("(k p) m -> p k m", p=P))

        def project_rope(src, w_sb, dst, ic):
            """dst[:, ic, :] = rope(src @ w_sb[:, ic-chunk]), per ns-half."""
            for ns in range(2):
                nsl = slice(ns * 512, (ns + 1) * 512)
                ps = psP.tile([P, 512], f32, tag="pj")
                for k in range(8):
                    nc.tensor.matmul(
                        ps[:],
                        lhsT=w_sb[:, k, ic * P:(ic + 1) * P],
                        rhs=src[:, k, nsl],
                        start=(k == 0),
                        stop=(k == 7),
                    )
                # rotate-strips and cos-mul read the PSUM directly: walrus
                # requires all SBUF operands of a TensorTensor to share the
                # same start partition, but PSUM operands are exempt — so the
                # cross-partition read must come from PSUM.
                tmp = tmpp.tile([P, 512], bf16, tag="tmp")
                for blk in range(4):
                    d0 = blk * 32
                    s0 = (blk ^ 1) * 32
                    nc.vector.tensor_mul(
                        out=tmp[d0:d0 + 32, :],
                        in0=ps[s0:s0 + 32, :],
                        in1=sin_sb[d0:d0 + 32, nsl],
                    )
                dv = dst[:, ic, nsl]
                nc.vector.tensor_mul(out=dv, in0=ps[:], in1=cos_sb[:, nsl])
                nc.gpsimd.tensor_add(out=dv, in0=dv, in1=tmp[:])

        qT = qk.tile([P, 4, N], bf16, tag="qT")
        kT = qk.tile([P, 4, N], bf16, tag="kT")
        aoT = qk.tile([P, 4, N], bf16, tag="aoT")

        def dots_exp(h, mch):
            """One m-chunk of exp(scale * k q^T) for head h -> [128,1024]."""
            t2, r0 = h // 2, (h % 2) * 64
            psd = psD.tile([P, N], f32, tag="dt")
            for ns in range(2):
                nc.tensor.matmul(
                    psd[:, ns * 512:(ns + 1) * 512],
                    lhsT=kT[r0:r0 + 64, t2, mch * P:(mch + 1) * P],
                    rhs=qT[r0:r0 + 64, t2, ns * 512:(ns + 1) * 512],
                    start=True,
                    stop=True,
                )
            e = epool.tile([P, N], bf16, tag="e")
            nc.scalar.activation(e[:], psd[:], Exp, scale=SCALE)
            return e

        def attn_v(h, es, h_ahead=None):
            """attn @ v for head h, interleaved per-mch with dots for head
            h_ahead (keeps PE busy while Act drains the dots psums; also
            matches the 24-slot es rotation: es[h][mch] is read by both
            attnv matmuls before dots(h_ahead, mch) overwrites its slot).
            Returns the deferred normalize closure."""
            t2, r0 = h // 2, (h % 2) * 64
            # pos tiles alternate between psB and psP (idle during attention)
            # so attnv(h) only waits on norm(h-2) — hides the denominator
            # DMA-bounce latency under two full head-blocks.
            pp, ptag = (psB, "av") if h % 2 == 0 else (psP, "pj")
            pos = [pp.tile([DH + 1, 512], f32, tag=ptag, name=f"po{_i}")
                   for _i in range(2)]
            for mch in range(8):
                for ns in range(2):
                    nc.tensor.matmul(
                        pos[ns][:],
                        lhsT=vsb[mch][:, h, :],
                        rhs=es[mch][:, ns * 512:(ns + 1) * 512],
                        start=(mch == 0),
                        stop=(mch == 7),
                    )
            es_ahead = ([dots_exp(h_ahead, mch) for mch in range(8)]
                        if h_ahead is not None else [])
            # reciprocal of softmax denominators straight off the psum row,
            # then a gpsimd partition-broadcast across the head's 64 rows
            # (broadcast must target base partition 0 on real HW; writing all
            # 128 rows costs the same since engine time is free-size only)
            rr = rrp.tile([1, N], f32, tag="rr")
            rb = rbcp.tile([P, N], f32, tag="rb")
            for ns in range(2):
                nsl = slice(ns * 512, (ns + 1) * 512)
                nc.vector.reciprocal(out=rr[:, nsl], in_=pos[ns][DH:DH + 1, :])
                nc.gpsimd.partition_broadcast(rb[:, nsl], rr[:, nsl])

            def _norm(pos=pos, rb=rb, t2=t2, r0=r0):
                for ns in range(2):
                    nsl = slice(ns * 512, (ns + 1) * 512)
                    nc.vector.tensor_mul(
                        out=aoT[r0:r0 + 64, t2, nsl], in0=pos[ns][0:DH, :],
                        in1=rb[r0:r0 + 64, nsl],
                    )
            return _norm, es_ahead

        # ---- interleaved k/q projections (+rope) so head-pair ic unblocks
        # as early as possible; heads 0-2 dots as soon as their chunks land
        project_rope(xT, wq_sb, qT, 0)
        project_rope(cT, wk_sb, kT, 0)
        es_list = [[dots_exp(0, mch) for mch in range(8)],
                   [dots_exp(1, mch) for mch in range(8)]]
        project_rope(xT, wq_sb, qT, 1)
        project_rope(cT, wk_sb, kT, 1)
        es_list.append([dots_exp(2, mch) for mch in range(8)])
        for ic in range(2, 4):
            project_rope(xT, wq_sb, qT, ic)
            project_rope(cT, wk_sb, kT, ic)

        # ---- v projections, ones-column for softmax denominators
        vsb = []
        for mch in range(8):
            psv = psP.tile([P, 512], f32, tag="pj")
            for k in range(8):
                nc.tensor.matmul(
                    psv[:],
                    lhsT=cT[:, k, mch * P:(mch + 1) * P],
                    rhs=wv_sb[:, k, :],
                    start=(k == 0),
                    stop=(k == 7),
                )
            vt = vpool.tile([P, 8, DH + 1], bf16, tag="v")
            nc.vector.tensor_copy(
                out=vt[:, :, 0:DH],
                in_=psv.rearrange("p (h d) -> p h d", d=DH),
            )
            nc.gpsimd.memset(vt[:, :, DH], 1.0)
            vsb.append(vt)

        # ---- attention pipeline: dots issued two heads ahead (24-buf epool),
        # normalize popped one head behind (hides the denominator DMA bounce),
        # out-projection round A (head-pairs 0-2) slotted in before head 7.
        def outproj_a():
            opart = []
            for nch in range(8):
                pso = psD.tile([P, N], f32, tag="dt")
                for cc in range(2):
                    for kc in range(3):
                        nc.tensor.matmul(
                            pso[:, cc * 512:(cc + 1) * 512],
                            lhsT=aoT[:, kc, nch * P:(nch + 1) * P],
                            rhs=wo_sb[:, kc, cc * 512:(cc + 1) * 512],
                            start=(kc == 0),
                            stop=(kc == 2),
                        )
                ot = osbp.tile([P, N], bf16, tag="opart")
                nc.vector.tensor_copy(out=ot[:], in_=pso[:])
                opart.append(ot)
            return opart

        pending = []
        for h in range(8):
            if pending:
                pending.pop(0)()
            m, es_ahead = attn_v(h, es_list[h], h + 3 if h + 3 <= 7 else None)
            pending.append(m)
            if es_ahead:
                es_list.append(es_ahead)
            if h == 6:
                opart = outproj_a()
        for m in pending:
            m()

        # ---- out-projection round B: head-pair 3 + add partial, DMA out
        for nch in range(8):
            pso = psD.tile([P, N], f32, tag="dt")
            for cc in range(2):
                nc.tensor.matmul(
                    pso[:, cc * 512:(cc + 1) * 512],
                    lhsT=aoT[:, 3, nch * P:(nch + 1) * P],
                    rhs=wo_sb[:, 3, cc * 512:(cc + 1) * 512],
                    start=True,
                    stop=True,
                )
            of = ofp.tile([P, N], bf16, tag="of")
            nc.vector.tensor_add(out=of[:], in0=pso[:], in1=opart[nch][:])
            nc.scalar.dma_start(out[nch * P:(nch + 1) * P, :], of[:])

    nc.compile()
    return nc


def _get_program():
    if "nc" not in _CACHE:
        _CACHE["nc"] = _build_program()
    return _CACHE["nc"]


def make_in_maps(x, context, rotary_pos, Wq, Wkv, Wout):
    from ml_dtypes import bfloat16

    x = np.asarray(x, dtype=np.float32)
    context = np.asarray(context, dtype=np.float32)
    rotary_pos = np.asarray(rotary_pos, dtype=np.float32)
    Wq = np.asarray(Wq, dtype=np.float32)
    Wkv = np.asarray(Wkv, dtype=np.float32)
    Wout = np.asarray(Wout, dtype=np.float32)

    cosT = np.ascontiguousarray(np.cos(rotary_pos).T)  # [64, 1024]
    sinT = np.sin(rotary_pos).T
    sin_signed = np.concatenate([-sinT[:32], sinT[32:]], axis=0)
    cos2 = np.vstack([cosT, cosT]).astype(bfloat16)
    sin2 = np.vstack([sin_signed, sin_signed]).astype(bfloat16)

    bc = lambda a: np.ascontiguousarray(a).astype(bfloat16)
    in_maps = []
    for core in range(8):
        b, g = core // 2, core % 2
        cs = slice(g * ISH, (g + 1) * ISH)
        in_maps.append({
            "xbT": bc(x[b].T),
            "cxT": bc(context[b].T),
            "wq": bc(Wq[:, cs]),
            "wk": bc(Wkv[:, g * ISH:(g + 1) * ISH]),
            "wv": bc(Wkv[:, H * DH + g * ISH:H * DH + (g + 1) * ISH]),
            "wo": bc(Wout[cs, :]),
            "cos2": cos2,
            "sin2": sin2,
        })
    return in_maps


def kernel(x, context, mask, context_mask, rotary_pos, Wq, Wkv, Wout, bout):
    global _LAST_EXEC_NS
    from concourse.bass_utils import run_bass_kernel_spmd

    nc = _get_program()
    in_maps = make_in_maps(x, context, rotary_pos, Wq, Wkv, Wout)

    trace = bool(os.environ.get("BASS_KERNEL_TRACE"))
    res = run_bass_kernel_spmd(nc, in_maps, core_ids=list(range(8)), trace=trace)
    _LAST_EXEC_NS = res.exec_time_ns
    _CACHE["last_results"] = res

    bout = np.asarray(bout, dtype=np.float32)
    full = np.empty((B, N, DIM), dtype=np.float32)
    for b in range(B):
        full[b] = (np.asarray(res.results[2 * b]["out"], dtype=np.float32)
                   + np.asarray(res.results[2 * b + 1]["out"], dtype=np.float32)
                   + bout)
    return full


# revision 46
# speedup vs baseline: 1.2532x; 1.0414x over previous
"""CrossAttention Trainium2 kernel (v2, all-bf16 single-pass pipeline).

Problem: nn_CrossAttention (B=4, N=M=1024, DIM=CTX_DIM=1024, H=16, DH=64).

Sharding: 8 cores = batch (4) x head-group (2 groups of 8 heads).
Each core computes, for its (b, g):
    q = rope(x[b] @ Wq[:, g])
    k = rope(context[b] @ Wk[:, g]);  v = context[b] @ Wv[:, g]
    attn = softmax(q k^T / sqrt(dh))     (mask is all-ones by construction)
    partial_out[b,g] = (attn @ v) @ Wout[g, :]
Host transposes x/context per batch, casts everything to bf16, sums the two
head-group partials per batch in fp32, and adds bout.

All tensors bf16 on SBUF (fp32 PSUM accumulation). Cost-model-driven layout:
engine time is free-dim-size only, DVE gets 2x for all-SBUF bf16 ops, PSUM
reads run 1x, activation exp is dtype-agnostic. Therefore:
  - dots psum tiles are 2-bank [128,1024] wide so each exp covers 1024 cols
  - rope copies PSUM->SBUF bf16 once (1x), then runs the 4 rotate-strip muls
    and cos-mul in bf16 SBUF (2x); the final add runs on the idle GPSIMD pool
  - softmax denominators: ones-column in V accumulates sum(exp) in psum row
    64; reciprocal runs directly on that psum row, then a DRAM bounce
    broadcasts it across the head's 64 partitions
  - out-projection is split: head-pairs 0-2 are projected mid-attention into
    bf16 SBUF partials, the tail only runs head-pair 3 + one add per n-chunk
"""

import os
import numpy as np

B, N, M = 4, 1024, 1024
DIM = 1024
H, DH = 16, 64
ISH = 512  # inner shard per core (8 heads * 64)
SCALE = DH ** -0.5
P = 128

_CACHE = {}
_LAST_EXEC_NS = None


def _build_program():
    from contextlib import ExitStack

    import concourse.tile as tile
    from concourse import bacc, mybir

    f32 = mybir.dt.float32
    bf16 = mybir.dt.bfloat16
    Exp = mybir.ActivationFunctionType.Exp

    nc = bacc.Bacc("TRN2", target_bir_lowering=False, debug=False, num_devices=8)

    xbT = nc.dram_tensor("xbT", [DIM, N], bf16, kind="ExternalInput").ap()
    cxT = nc.dram_tensor("cxT", [DIM, M], bf16, kind="ExternalInput").ap()
    wq = nc.dram_tensor("wq", [DIM, ISH], bf16, kind="ExternalInput").ap()
    wk = nc.dram_tensor("wk", [DIM, ISH], bf16, kind="ExternalInput").ap()
    wv = nc.dram_tensor("wv", [DIM, ISH], bf16, kind="ExternalInput").ap()
    wo = nc.dram_tensor("wo", [ISH, DIM], bf16, kind="ExternalInput").ap()
    cos2 = nc.dram_tensor("cos2", [P, N], bf16, kind="ExternalInput").ap()
    sin2 = nc.dram_tensor("sin2", [P, N], bf16, kind="ExternalInput").ap()
    out = nc.dram_tensor("out", [N, DIM], bf16, kind="ExternalOutput").ap()

    with tile.TileContext(nc) as tc, ExitStack() as ctx:
        const = ctx.enter_context(tc.tile_pool(name="const", bufs=1))
        wpool = ctx.enter_context(tc.tile_pool(name="wpool", bufs=1))
        xpool = ctx.enter_context(tc.tile_pool(name="xpool", bufs=1))
        qk = ctx.enter_context(tc.tile_pool(name="qk", bufs=1))
        qbp = ctx.enter_context(tc.tile_pool(name="qbp", bufs=3))
        tmpp = ctx.enter_context(tc.tile_pool(name="tmpp", bufs=3))
        vpool = ctx.enter_context(tc.tile_pool(name="vpool", bufs=8))
        epool = ctx.enter_context(tc.tile_pool(name="epool", bufs=24))
        rrp = ctx.enter_context(tc.tile_pool(name="rrp", bufs=2))
        rbcp = ctx.enter_context(tc.tile_pool(name="rbcp", bufs=2))
        osbp = ctx.enter_context(tc.tile_pool(name="osbp", bufs=8))
        ofp = ctx.enter_context(tc.tile_pool(name="ofp", bufs=4))
        psP = ctx.enter_context(tc.tile_pool(name="psP", bufs=2, space="PSUM"))
        psD = ctx.enter_context(tc.tile_pool(name="psD", bufs=2, space="PSUM"))
        psB = ctx.enter_context(tc.tile_pool(name="psB", bufs=2, space="PSUM"))

        cos_sb = const.tile([P, N], bf16, tag="cos")
        sin_sb = const.tile([P, N], bf16, tag="sin")

        # ---- input loads: one big rearranged DMA per tensor (per-DMA fixed
        # overheads dominate chunked loads; every projection contracts over
        # all 8 chunks anyway so chunk-granular gating buys nothing).
        # preload the Exp activation table while DMAs run (takes 1.3us; off
        # the first-exp critical path this way)
        dummy = const.tile([1, 8], f32, tag="dummy")
        nc.vector.memset(dummy[:], 0.0)
        nc.scalar.activation(dummy[:], dummy[:], Exp, scale=1.0)

        wq_sb = wpool.tile([P, 8, ISH], bf16, tag="wq")
        nc.sync.dma_start(wq_sb[:], wq.rearrange("(k p) m -> p k m", p=P))
        xT = xpool.tile([P, 8, N], bf16, tag="xT")
        nc.sync.dma_start(xT[:], xbT.rearrange("(k p) n -> p k n", p=P))
        wk_sb = wpool.tile([P, 8, ISH], bf16, tag="wk")
        nc.sync.dma_start(wk_sb[:], wk.rearrange("(k p) m -> p k m", p=P))
        cT = xpool.tile([P, 8, N], bf16, tag="cT")
        nc.sync.dma_start(cT[:], cxT.rearrange("(k p) n -> p k n", p=P))
        # smaller late-needed loads issued after the critical four so their
        # transfers don't delay x/ctx on the (serialized) DMA engines
        nc.gpsimd.dma_start(cos_sb[:], cos2)
        nc.gpsimd.dma_start(sin_sb[:], sin2)
        wv_sb = wpool.tile([P, 8, ISH], bf16, tag="wv")
        nc.gpsimd.dma_start(wv_sb[:], wv.rearrange("(k p) m -> p k m", p=P))
        wo_sb = wpool.tile([P, 4, DIM], bf16, tag="wo")
        nc.gpsimd.dma_start(wo_sb[:], wo.rearrange("(k p) m -> p k m", p=P))

        def project_rope(src, w_sb, dst, ic, pools=None):
            """dst[:, ic, :] = rope(src @ w_sb[:, ic-chunk]), per ns-half."""
            for ns in range(2):
                nsl = slice(ns * 512, (ns + 1) * 512)
                pp, ptag = (pools or ((psP, "pj"), (psB, "av")))[ns]
                ps = pp.tile([P, 512], f32, tag=ptag)
                for k in range(8):
                    nc.tensor.matmul(
                        ps[:],
                        lhsT=w_sb[:, k, ic * P:(ic + 1) * P],
                        rhs=src[:, k, nsl],
                        start=(k == 0),
                        stop=(k == 7),
                    )
                # rotate-strips and cos-mul read the PSUM directly: walrus
                # requires all SBUF operands of a TensorTensor to share the
                # same start partition, but PSUM operands are exempt — so the
                # cross-partition read must come from PSUM.
                tmp = tmpp.tile([P, 512], bf16, tag="tmp")
                for blk in range(4):
                    d0 = blk * 32
                    s0 = (blk ^ 1) * 32
                    nc.vector.tensor_mul(
                        out=tmp[d0:d0 + 32, :],
                        in0=ps[s0:s0 + 32, :],
                        in1=sin_sb[d0:d0 + 32, nsl],
                    )
                dv = dst[:, ic, nsl]
                nc.vector.tensor_mul(out=dv, in0=ps[:], in1=cos_sb[:, nsl])
                nc.gpsimd.tensor_add(out=dv, in0=dv, in1=tmp[:])

        qT = qk.tile([P, 4, N], bf16, tag="qT")
        kT = qk.tile([P, 4, N], bf16, tag="kT")
        aoT = qk.tile([P, 4, N], bf16, tag="aoT")

        def dots_exp(h, mch):
            """One m-chunk of exp(scale * k q^T) for head h -> [128,1024]."""
            t2, r0 = h // 2, (h % 2) * 64
            psd = psD.tile([P, N], f32, tag="dt")
            for ns in range(2):
                nc.tensor.matmul(
                    psd[:, ns * 512:(ns + 1) * 512],
                    lhsT=kT[r0:r0 + 64, t2, mch * P:(mch + 1) * P],
                    rhs=qT[r0:r0 + 64, t2, ns * 512:(ns + 1) * 512],
                    start=True,
                    stop=True,
                )
            e = epool.tile([P, N], bf16, tag="e")
            nc.scalar.activation(e[:], psd[:], Exp, scale=SCALE)
            return e

        def attn_v(h, es, h_ahead=None):
            """attn @ v for head h, interleaved per-mch with dots for head
            h_ahead (keeps PE busy while Act drains the dots psums; also
            matches the 24-slot es rotation: es[h][mch] is read by both
            attnv matmuls before dots(h_ahead, mch) overwrites its slot).
            Returns the deferred normalize closure."""
            t2, r0 = h // 2, (h % 2) * 64
            pos = [psB.tile([DH + 1, 512], f32, tag="av", name=f"po{_i}")
                   for _i in range(2)]
            for mch in range(8):
                for ns in range(2):
                    nc.tensor.matmul(
                        pos[ns][:],
                        lhsT=vsb[mch][:, h, :],
                        rhs=es[mch][:, ns * 512:(ns + 1) * 512],
                        start=(mch == 0),
                        stop=(mch == 7),
                    )
            es_ahead = ([dots_exp(h_ahead, mch) for mch in range(8)]
                        if h_ahead is not None else [])
            # reciprocal of softmax denominators straight off the psum row,
            # then a gpsimd partition-broadcast across the head's 64 rows
            # (broadcast must target base partition 0 on real HW; writing all
            # 128 rows costs the same since engine time is free-size only)
            rr = rrp.tile([1, N], f32, tag="rr")
            rb = rbcp.tile([P, N], f32, tag="rb")
            for ns in range(2):
                nsl = slice(ns * 512, (ns + 1) * 512)
                nc.vector.reciprocal(out=rr[:, nsl], in_=pos[ns][DH:DH + 1, :])
                nc.gpsimd.partition_broadcast(rb[:, nsl], rr[:, nsl])

            def _norm(pos=pos, rb=rb, t2=t2, r0=r0):
                for ns in range(2):
                    nsl = slice(ns * 512, (ns + 1) * 512)
                    nc.vector.tensor_mul(
                        out=aoT[r0:r0 + 64, t2, nsl], in0=pos[ns][0:DH, :],
                        in1=rb[r0:r0 + 64, nsl],
                    )
            return _norm, es_ahead

        # ---- q/k projections for head-pairs 0-1 (+rope), dots for heads 0-2
        # up front; ic2/ic3 projections are injected into the head loop so
        # their ropes overlap attention instead of delaying attnv(h0).
        project_rope(xT, wq_sb, qT, 0)
        project_rope(cT, wk_sb, kT, 0)
        es_list = [[dots_exp(0, mch) for mch in range(8)],
                   [dots_exp(1, mch) for mch in range(8)]]
        project_rope(xT, wq_sb, qT, 1)
        project_rope(cT, wk_sb, kT, 1)
        es_list.append([dots_exp(2, mch) for mch in range(8)])

        # ---- v projections, ones-column for softmax denominators
        vsb = []
        for mch in range(8):
            psv = psP.tile([P, 512], f32, tag="pj")
            for k in range(8):
                nc.tensor.matmul(
                    psv[:],
                    lhsT=cT[:, k, mch * P:(mch + 1) * P],
                    rhs=wv_sb[:, k, :],
                    start=(k == 0),
                    stop=(k == 7),
                )
            vt = vpool.tile([P, 8, DH + 1], bf16, tag="v")
            nc.vector.tensor_copy(
                out=vt[:, :, 0:DH],
                in_=psv.rearrange("p (h d) -> p h d", d=DH),
            )
            nc.gpsimd.memset(vt[:, :, DH], 1.0)
            vsb.append(vt)

        # ---- attention pipeline: dots issued two heads ahead (24-buf epool),
        # normalize popped one head behind (hides the denominator DMA bounce),
        # out-projection round A (head-pairs 0-2) slotted in before head 7.
        def outproj_a():
            opart = []
            for nch in range(8):
                pso = psD.tile([P, N], f32, tag="dt")
                for cc in range(2):
                    for kc in range(3):
                        nc.tensor.matmul(
                            pso[:, cc * 512:(cc + 1) * 512],
                            lhsT=aoT[:, kc, nch * P:(nch + 1) * P],
                            rhs=wo_sb[:, kc, cc * 512:(cc + 1) * 512],
                            start=(kc == 0),
                            stop=(kc == 2),
                        )
                ot = osbp.tile([P, N], bf16, tag="opart")
                nc.vector.tensor_copy(out=ot[:], in_=pso[:])
                opart.append(ot)
            return opart

        # Injections sit between the norm-pop and attn_v so they never land
        # between recip(h) and norm(h) on the in-order DVE queue (which would
        # delay norm and stall the next head's psum-slot rotation).
        pp_both = ((psP, "pj"), (psP, "pj"))
        pending = []
        for h in range(8):
            if pending:
                pending.pop(0)()
            m, es_ahead = attn_v(h, es_list[h], h + 3 if h + 3 <= 7 else None)
            pending.append(m)
            if es_ahead:
                es_list.append(es_ahead)
            if h == 0:
                project_rope(xT, wq_sb, qT, 2, pools=pp_both)
                project_rope(cT, wk_sb, kT, 2, pools=pp_both)
            elif h == 1:
                project_rope(xT, wq_sb, qT, 3, pools=pp_both)
                project_rope(cT, wk_sb, kT, 3, pools=pp_both)
            elif h == 6:
                opart = outproj_a()
        for m in pending:
            m()

        # ---- out-projection round B: head-pair 3 + add partial, DMA out
        for nch in range(8):
            pso = psD.tile([P, N], f32, tag="dt")
            for cc in range(2):
                nc.tensor.matmul(
                    pso[:, cc * 512:(cc + 1) * 512],
                    lhsT=aoT[:, 3, nch * P:(nch + 1) * P],
                    rhs=wo_sb[:, 3, cc * 512:(cc + 1) * 512],
                    start=True,
                    stop=True,
                )
            of = ofp.tile([P, N], bf16, tag="of")
            nc.vector.tensor_add(out=of[:], in0=pso[:], in1=opart[nch][:])
            nc.scalar.dma_start(out[nch * P:(nch + 1) * P, :], of[:])

    nc.compile()
    return nc


def _get_program():
    if "nc" not in _CACHE:
        _CACHE["nc"] = _build_program()
    return _CACHE["nc"]


def make_in_maps(x, context, rotary_pos, Wq, Wkv, Wout):
    from ml_dtypes import bfloat16

    x = np.asarray(x, dtype=np.float32)
    context = np.asarray(context, dtype=np.float32)
    rotary_pos = np.asarray(rotary_pos, dtype=np.float32)
    Wq = np.asarray(Wq, dtype=np.float32)
    Wkv = np.asarray(Wkv, dtype=np.float32)
    Wout = np.asarray(Wout, dtype=np.float32)

    cosT = np.ascontiguousarray(np.cos(rotary_pos).T)  # [64, 1024]
    sinT = np.sin(rotary_pos).T
    sin_signed = np.concatenate([-sinT[:32], sinT[32:]], axis=0)
    cos2 = np.vstack([cosT, cosT]).astype(bfloat16)
    sin2 = np.vstack([sin_signed, sin_signed]).astype(bfloat16)

    bc = lambda a: np.ascontiguousarray(a).astype(bfloat16)
    in_maps = []
    for core in range(8):
        b, g = core // 2, core % 2
        cs = slice(g * ISH, (g + 1) * ISH)
        in_maps.append({
            "xbT": bc(x[b].T),
            "cxT": bc(context[b].T),
            "wq": bc(Wq[:, cs]),
            "wk": bc(Wkv[:, g * ISH:(g + 1) * ISH]),
            "wv": bc(Wkv[:, H * DH + g * ISH:H * DH + (g + 1) * ISH]),
            "wo": bc(Wout[cs, :]),
            "cos2": cos2,
            "sin2": sin2,
        })
    return in_maps


def kernel(x, context, mask, context_mask, rotary_pos, Wq, Wkv, Wout, bout):
    global _LAST_EXEC_NS
    from concourse.bass_utils import run_bass_kernel_spmd

    nc = _get_program()
    in_maps = make_in_maps(x, context, rotary_pos, Wq, Wkv, Wout)

    trace = bool(os.environ.get("BASS_KERNEL_TRACE"))
    res = run_bass_kernel_spmd(nc, in_maps, core_ids=list(range(8)), trace=trace)
    _LAST_EXEC_NS = res.exec_time_ns
    _CACHE["last_results"] = res

    bout = np.asarray(bout, dtype=np.float32)
    full = np.empty((B, N, DIM), dtype=np.float32)
    for b in range(B):
        full[b] = (np.asarray(res.results[2 * b]["out"], dtype=np.float32)
                   + np.asarray(res.results[2 * b + 1]["out"], dtype=np.float32)
                   + bout)
    return full
